# revision 3
# baseline (speedup 1.0000x reference)
"""BnBinActiveConv2d Trainium2 kernel.

Pipeline (per reference):
  BN (batch stats, train mode) -> BinActive (sign + K = box(mean_c |xn|))
  -> BinConv (sign weights) -> relu(y * K * alpha)

Math used by this kernel (gamma > 0 assumed; gamma==1, beta==0 in practice):
  s_c = gamma_c * rsqrt(var_c + eps),  t_c = mu_c - beta_c / s_c
  sign(xn) = sign(x - t_c)
  |xn|     = |s_c * x - s_c * t_c|
  out      = relu(alpha_co * y) * K,  K folded as raw box-sum with the
             1/(9C) normalization folded into alpha.

Sharding: data-parallel over batch, 4 images per core on 8 cores. Only the
BN statistics (mean, E[x^2] per channel: 2KB) are all-reduced across cores.

The conv is 9 shifted fp8e4m3 DoubleRow matmuls per output tile (one per
tap, contracting all 256 input channels as 128 partitions x 2) accumulated
in PSUM; sign values are exact in fp8 so sums are exact integers in fp32
PSUM. The rhs streams flat full-width row chunks (56 cols incl. 2 junk
cols per row, dropped at PSUM evacuation) to keep the DoubleRow AP 3D.
A bf16 fallback path (fp8=False) does 18 matmuls per tile with 4D APs.
"""

import numpy as np
from contextlib import ExitStack

import concourse.bass as bass
import concourse.bacc as bacc
import concourse.tile as tile
from concourse import mybir
from concourse.alu_op_type import AluOpType

AFT = mybir.ActivationFunctionType
FP32 = mybir.dt.float32
BF16 = mybir.dt.bfloat16
FP8 = mybir.dt.float8e4

BN_EPS = 1e-4
P = 128


def _chunk(total, cap):
    """Largest divisor of `total` that is <= cap."""
    for c in range(min(cap, total), 0, -1):
        if total % c == 0:
            return c
    return total


def build(B_loc=4, C=256, H=56, KS=3, n_cores=8, fp8=False, loop_k=1,
          stats_once=False, local=False):
    W_ = H
    HO, WO = H - KS + 1, W_ - KS + 1
    CC = C // P
    HW, HOWO = H * W_, HO * WO
    CKK = C * KS * KS

    G = _chunk(HO, 512 // WO)          # output rows per conv psum tile
    NG = HO // G                       # conv groups per image
    GF = _chunk(HO, 512 // W_)         # fp8 path: full-W rows per psum tile
    NGF = HO // GF
    HWP = HW + 64                      # padded per-chunk stride for fp8 xs
    NJ = _chunk(HW, 448)               # A-sum / bn_stats subgroup
    JN = HW // NJ

    nc = bacc.Bacc("TRN2", target_bir_lowering=False, debug=False,
                   enable_asserts=False, num_devices=n_cores)

    x_d = nc.dram_tensor("x", [B_loc, C, H, W_], FP32, kind="ExternalInput").ap()
    wt_d = nc.dram_tensor("wt", [C, KS, KS, C], FP32, kind="ExternalInput").ap()
    wo_d = nc.dram_tensor("wo", [C, CKK], FP32, kind="ExternalInput").ap()
    g_d = nc.dram_tensor("gamma", [C], FP32, kind="ExternalInput").ap()
    b_d = nc.dram_tensor("beta", [C], FP32, kind="ExternalInput").ap()
    y_d = nc.dram_tensor("y", [B_loc, C, HO, WO], FP32, kind="ExternalOutput").ap()

    with tile.TileContext(nc) as tc:
        with ExitStack() as ctx:
            consts = ctx.enter_context(tc.tile_pool(name="consts", bufs=1))
            statsp = ctx.enter_context(tc.tile_pool(name="stats", bufs=1))
            wbp = ctx.enter_context(tc.tile_pool(name="wb", bufs=1))
            dram = ctx.enter_context(tc.tile_pool(name="dram", bufs=1, space="DRAM"))

            # ---- small constants ----
            ones_bf = consts.tile([P, 1], BF16)
            nc.vector.memset(ones_bf, 1.0)
            gam = consts.tile([P, CC], FP32)
            nc.sync.dma_start(out=gam, in_=g_d.rearrange("(cc p) -> p cc", p=P))
            bet = consts.tile([P, CC], FP32)
            nc.sync.dma_start(out=bet, in_=b_d.rearrange("(cc p) -> p cc", p=P))

            alpha_s = consts.tile([P, CC], FP32)   # alpha / (C*KS^2), per co
            neg_t = consts.tile([P, CC], FP32)     # -t_c per ci
            nst = consts.tile([P, CC], FP32)       # -s_c * t_c per ci
            s_sb = consts.tile([P, CC], FP32)      # s_c per ci

            # ---- weights: sign(W) as lhsT layout [ci, (kh kw co)], alpha ----
            if fp8:
                assert CC == 2
                wb8 = wbp.tile([P, CC, KS * KS * C], FP8, name="wb8")
            else:
                wb = [wbp.tile([P, KS * KS * C], BF16, tag=f"wb{cc}", name=f"wb{cc}")
                      for cc in range(CC)]
            with tc.tile_pool(name="wtmp", bufs=2) as wtmp:
                for cc in range(CC):
                    wt_f = wtmp.tile([P, KS * KS * C], FP32, tag="wtmp")
                    nc.sync.dma_start(
                        out=wt_f,
                        in_=wt_d[cc * P:(cc + 1) * P].rearrange("c a b o -> c (a b o)"))
                    if fp8:
                        nc.scalar.activation(out=wb8[:, cc, :], in_=wt_f, func=AFT.Sign)
                    else:
                        nc.scalar.activation(out=wb[cc], in_=wt_f, func=AFT.Sign)
                for cc in range(CC):
                    wo_f = wtmp.tile([P, CKK], FP32, tag="wtmp")
                    nc.sync.dma_start(out=wo_f, in_=wo_d[cc * P:(cc + 1) * P])
                    nc.vector.tensor_reduce(
                        out=alpha_s[:, cc:cc + 1], in_=wo_f,
                        axis=mybir.AxisListType.X, op=AluOpType.add,
                        apply_absolute_value=True)
            # alpha_s = abs_sum / CKK (-> alpha) / CKK (-> fold 1/(C*KS^2) for K)
            nc.vector.tensor_scalar_mul(alpha_s, alpha_s, 1.0 / (CKK * CKK))

            xpool = ctx.enter_context(tc.tile_pool(name="x", bufs=1))
            xsp = ctx.enter_context(tc.tile_pool(name="xs", bufs=2 * CC))
            adp = ctx.enter_context(tc.tile_pool(name="ad", bufs=2 * CC))
            arp = ctx.enter_context(tc.tile_pool(name="arow", bufs=1))
            boxp = ctx.enter_context(tc.tile_pool(name="box", bufs=2))
            kbcp = ctx.enter_context(tc.tile_pool(name="kbc", bufs=2))
            yp = ctx.enter_context(tc.tile_pool(name="y", bufs=4))
            psA = ctx.enter_context(tc.tile_pool(name="psA", bufs=2, space="PSUM"))
            psC = ctx.enter_context(tc.tile_pool(name="psC", bufs=4, space="PSUM"))
            araw_d = dram.tile([B_loc, HW], FP32)
            kflat_d = dram.tile([B_loc, HOWO], FP32)
            cc_in = dram.tile([P, 2 * CC], FP32)
            cc_out = dram.tile([P, 2 * CC], FP32)
            eps_sb = statsp.tile([P, 1], FP32)
            nc.vector.memset(eps_sb, BN_EPS)

            for rep in range(loop_k):
                # ---- phase 1: load x, per-core BN partial stats ----
                x_sb = {}
                stats = [statsp.tile([P, B_loc * JN, 6], FP32, tag=f"st{cc}", name=f"st{cc}r{rep}")
                         for cc in range(CC)]
                for n in range(B_loc):
                    for cc in range(CC):
                        xt = xpool.tile([P, HW], FP32, tag=f"x{n}{cc}", name=f"xt{n}{cc}r{rep}")
                        nc.sync.dma_start(
                            out=xt,
                            in_=x_d[n, cc * P:(cc + 1) * P].rearrange("c h w -> c (h w)"))
                        x_sb[n, cc] = xt
                        xr = xt.rearrange("p (j v) -> p j v", v=NJ)
                        for j in range(JN):
                            nc.vector.bn_stats(out=stats[cc][:, n * JN + j, :],
                                               in_=xr[:, j, :])

                if rep == 0 or not stats_once:
                    # ---- phase 2: all-reduce stats, derive s, t ----
                    pk = statsp.tile([P, 2 * CC], FP32)
                    for cc in range(CC):
                        mv = statsp.tile([P, 2], FP32, tag="mv")
                        nc.vector.bn_aggr(out=mv, in_=stats[cc])
                        nc.vector.tensor_copy(out=pk[:, 2 * cc:2 * cc + 1], in_=mv[:, 0:1])
                        # m2 = var + mean^2
                        msq = statsp.tile([P, 1], FP32, tag="msq")
                        nc.vector.tensor_tensor(out=msq, in0=mv[:, 0:1], in1=mv[:, 0:1],
                                                op=AluOpType.mult)
                        nc.vector.tensor_tensor(out=pk[:, 2 * cc + 1:2 * cc + 2],
                                                in0=msq, in1=mv[:, 1:2], op=AluOpType.add)
                    nc.sync.dma_start(out=cc_in, in_=pk)
                    if local:
                        nc.sync.dma_start(out=cc_out, in_=cc_in)
                    else:
                        nc.gpsimd.collective_compute(
                            "AllReduce", AluOpType.add,
                            replica_groups=[list(range(n_cores))],
                            ins=[cc_in.opt()], outs=[cc_out.opt()])
                    sums = statsp.tile([P, 2 * CC], FP32)
                    nc.sync.dma_start(out=sums, in_=cc_out)
                    nc.vector.tensor_scalar_mul(sums, sums, 1.0 / n_cores)

                    for cc in range(CC):
                        mean = sums[:, 2 * cc:2 * cc + 1]
                        ex2 = sums[:, 2 * cc + 1:2 * cc + 2]
                        var = statsp.tile([P, 1], FP32, tag="var")
                        nc.vector.tensor_tensor(out=var, in0=mean, in1=mean, op=AluOpType.mult)
                        nc.vector.tensor_tensor(out=var, in0=ex2, in1=var, op=AluOpType.subtract)
                        r = statsp.tile([P, 1], FP32, tag="r")
                        nc.scalar.activation(out=r, in_=var, func=AFT.Sqrt, bias=eps_sb)
                        nc.vector.reciprocal(out=r, in_=r)
                        # two Newton steps: r <- r * (1.5 - 0.5 * (var+eps) * r^2)
                        ve = statsp.tile([P, 1], FP32, tag="ve")
                        nc.vector.tensor_scalar(out=ve, in0=var, scalar1=1.0, scalar2=BN_EPS,
                                                op0=AluOpType.mult, op1=AluOpType.add)
                        for _ in range(2):
                            rr = statsp.tile([P, 1], FP32, tag="rr")
                            nc.vector.tensor_tensor(out=rr, in0=r, in1=r, op=AluOpType.mult)
                            nc.vector.tensor_tensor(out=rr, in0=rr, in1=ve, op=AluOpType.mult)
                            nc.vector.tensor_scalar(out=rr, in0=rr, scalar1=-0.5, scalar2=1.5,
                                                    op0=AluOpType.mult, op1=AluOpType.add)
                            nc.vector.tensor_tensor(out=r, in0=r, in1=rr, op=AluOpType.mult)
                        nc.vector.tensor_tensor(out=s_sb[:, cc:cc + 1], in0=r,
                                                in1=gam[:, cc:cc + 1], op=AluOpType.mult)
                        inv_s = statsp.tile([P, 1], FP32, tag="invs")
                        nc.vector.reciprocal(out=inv_s, in_=s_sb[:, cc:cc + 1])
                        # neg_t = beta * (1/s) - mean
                        nc.vector.scalar_tensor_tensor(
                            out=neg_t[:, cc:cc + 1], in0=bet[:, cc:cc + 1], scalar=inv_s,
                            in1=mean, op0=AluOpType.mult, op1=AluOpType.subtract)
                        nc.vector.tensor_tensor(out=nst[:, cc:cc + 1],
                                                in0=s_sb[:, cc:cc + 1],
                                                in1=neg_t[:, cc:cc + 1], op=AluOpType.mult)

                # ---- phase 3: per image: binarize, A/K, conv, scale, store ----


                for n in range(B_loc):
                    xs, ad = {}, {}
                    if fp8:
                        xs8 = xsp.tile([P, CC, HWP], FP8, tag="xs", name=f"xs8{n}r{rep}")
                        nc.vector.memset(xs8[:, :, HW:HWP], 0.0)
                    for cc in range(CC):
                        if fp8:
                            nc.scalar.activation(out=xs8[:, cc, 0:HW], in_=x_sb[n, cc],
                                                 func=AFT.Sign, bias=neg_t[:, cc:cc + 1])
                        else:
                            xs[cc] = xsp.tile([P, HW], BF16, tag="xs", name=f"xs{n}{cc}r{rep}")
                            nc.scalar.activation(out=xs[cc], in_=x_sb[n, cc], func=AFT.Sign,
                                                 bias=neg_t[:, cc:cc + 1])
                        ad[cc] = adp.tile([P, HW], BF16, tag="ad", name=f"ad{n}{cc}r{rep}")
                        nc.scalar.activation(out=ad[cc], in_=x_sb[n, cc], func=AFT.Abs,
                                             bias=nst[:, cc:cc + 1],
                                             scale=s_sb[:, cc:cc + 1])

                    # A raw channel-sum: ones.T @ (s|d|)  -> [1, HW]
                    a_row = arp.tile([1, HW], FP32, tag="arow")
                    for j in range(JN):
                        pa = psA.tile([1, NJ], FP32, tag="psA")
                        for cc in range(CC):
                            nc.tensor.matmul(pa, lhsT=ones_bf,
                                             rhs=ad[cc][:, j * NJ:(j + 1) * NJ],
                                             start=(cc == 0), stop=(cc == CC - 1))
                        nc.vector.tensor_copy(out=a_row[:, j * NJ:(j + 1) * NJ], in_=pa)
                    nc.sync.dma_start(out=araw_d[n:n + 1, :], in_=a_row)

                    # box filter via 3 row-shifted reloads + free-dim shifts
                    av = araw_d[n, :].rearrange("(h w) -> h w", w=W_)
                    t1 = boxp.tile([HO, W_], FP32, tag="t1")
                    a_sh = [boxp.tile([HO, W_], FP32, tag=f"ash{k}", name=f"ash{n}{k}r{rep}")
                            for k in range(KS)]
                    for k in range(KS):
                        nc.sync.dma_start(out=a_sh[k], in_=av[k:k + HO, :])
                    nc.vector.tensor_tensor(out=t1, in0=a_sh[0], in1=a_sh[1],
                                            op=AluOpType.add)
                    for k in range(2, KS):
                        nc.vector.tensor_tensor(out=t1, in0=t1, in1=a_sh[k],
                                                op=AluOpType.add)
                    k_im = boxp.tile([HO, WO], FP32, tag="kim")
                    nc.vector.tensor_tensor(out=k_im, in0=t1[:, 0:WO], in1=t1[:, 1:WO + 1],
                                            op=AluOpType.add)
                    for k in range(2, KS):
                        nc.vector.tensor_tensor(out=k_im, in0=k_im, in1=t1[:, k:WO + k],
                                                op=AluOpType.add)
                    nc.sync.dma_start(
                        out=kflat_d[n, :].rearrange("(h w) -> h w", w=WO), in_=k_im)

                    # broadcast K to all partitions
                    k_bc = kbcp.tile([P, HOWO], FP32, tag="kbc")
                    ksrc = kflat_d[n, :]
                    nc.gpsimd.dma_start(
                        out=k_bc,
                        in_=bass.AP(tensor=ksrc.tensor, offset=ksrc.offset,
                                    ap=[[0, P]] + list(ksrc.ap)))

                    # conv + scale + store
                    if fp8:
                        wb8v = wb8.rearrange("p c (k o) -> p c k o", o=C)
                        for co in range(CC):
                            for g in range(NGF):
                                pc = psC.tile([P, GF, W_], FP32, tag="psC")
                                first = True
                                for kh in range(KS):
                                    for kw in range(KS):
                                        last = (kh == KS - 1 and kw == KS - 1)
                                        off = g * GF * W_ + kh * W_ + kw
                                        nc.tensor.matmul(
                                            pc,
                                            lhsT=wb8v[:, :, kh * KS + kw, co * P:(co + 1) * P],
                                            rhs=xs8[:, :, off:off + GF * W_],
                                            start=first, stop=last,
                                            perf_mode=mybir.MatmulPerfMode.DoubleRow)
                                        first = False
                                y_t = yp.tile([P, GF * WO], FP32, tag="y")
                                nc.scalar.activation(out=y_t.rearrange("p (g w) -> p g w", w=WO),
                                                     in_=pc[:, :, 0:WO],
                                                     func=AFT.Relu,
                                                     scale=alpha_s[:, co:co + 1])
                                nc.vector.tensor_tensor(
                                    out=y_t, in0=y_t,
                                    in1=k_bc[:, g * GF * WO:(g + 1) * GF * WO],
                                    op=AluOpType.mult)
                                nc.sync.dma_start(
                                    out=y_d[n, co * P:(co + 1) * P,
                                            g * GF:(g + 1) * GF, :].rearrange("c h w -> c (h w)"),
                                    in_=y_t)
                        continue
                    for co in range(CC):
                        for g in range(NG):
                            pc = psC.tile([P, G, WO], FP32, tag="psC")
                            first = True
                            if True:
                                wbv = [wb[cc].rearrange("p (k c) -> p k c", c=C)
                                       for cc in range(CC)]
                                for cc in range(CC):
                                    xsv = xs[cc].rearrange("p (h w) -> p h w", w=W_)
                                    for kh in range(KS):
                                        for kw in range(KS):
                                            last = (cc == CC - 1 and kh == KS - 1
                                                    and kw == KS - 1)
                                            nc.tensor.matmul(
                                                pc,
                                                lhsT=wbv[cc][:, kh * KS + kw,
                                                             co * P:(co + 1) * P],
                                                rhs=xsv[:, g * G + kh:g * G + kh + G,
                                                        kw:kw + WO],
                                                start=first, stop=last)
                                            first = False
                            y_t = yp.tile([P, G * WO], FP32, tag="y")
                            nc.scalar.activation(out=y_t,
                                                 in_=pc.rearrange("p g w -> p (g w)"),
                                                 func=AFT.Relu,
                                                 scale=alpha_s[:, co:co + 1])
                            nc.vector.tensor_tensor(
                                out=y_t, in0=y_t,
                                in1=k_bc[:, g * G * WO:(g + 1) * G * WO],
                                op=AluOpType.mult)
                            nc.sync.dma_start(
                                out=y_d[n, co * P:(co + 1) * P,
                                        g * G:(g + 1) * G, :].rearrange("c h w -> c (h w)"),
                                in_=y_t)

    nc.compile()
    return nc


_CACHE = {}


def _get_compiled():
    # fp8 DoubleRow conv: sign(+-1) values are exact in fp8e4m3, PSUM
    # accumulation is fp32, so results match the bf16 path bit-for-bit in
    # accuracy while halving TensorE work and instruction count.
    if "nc" not in _CACHE:
        _CACHE["nc"] = build(fp8=True)
    return _CACHE["nc"]


def make_in_maps(x, gamma, beta, W, n_cores=8):
    x = np.ascontiguousarray(np.asarray(x, dtype=np.float32))
    gamma = np.ascontiguousarray(np.asarray(gamma, dtype=np.float32))
    beta = np.ascontiguousarray(np.asarray(beta, dtype=np.float32))
    W = np.asarray(W, dtype=np.float32)
    wt = np.ascontiguousarray(np.transpose(W, (1, 2, 3, 0)))
    wo = np.ascontiguousarray(W.reshape(W.shape[0], -1))
    B_loc = x.shape[0] // n_cores
    return [
        {"x": np.ascontiguousarray(x[c * B_loc:(c + 1) * B_loc]),
         "wt": wt, "wo": wo, "gamma": gamma, "beta": beta}
        for c in range(n_cores)
    ]


def run(x, gamma, beta, W, trace=False):
    from concourse import bass_utils
    nc = _get_compiled()
    in_maps = make_in_maps(x, gamma, beta, W)
    res = bass_utils.run_bass_kernel_spmd(nc, in_maps, core_ids=list(range(8)),
                                          trace=trace)
    out = np.concatenate([r["y"] for r in res.results], axis=0)
    return out, res


def kernel(x, gamma, beta, W):
    out, _ = run(x, gamma, beta, W)
    return out

